# revision 6
# baseline (speedup 1.0000x reference)
"""Class-balanced cross-entropy loss kernel for Trainium2 (8 NeuronCores).

Problem: output [4,8,64,128,128] f32 logits, labels [4,1,64,128,128] int
(values 0..7).  loss = mean over present classes of (per-class mean CE).

Design ("plane layout", data-parallel over voxels, 524288 voxels/core):
  Host sends x as 8 class-planes in a voxel-major layout (bf16, 8.4MB) plus
  labels (bf16, 1MB).  No one-hot is shipped (it is generated on-device by
  4x-rate tensor_scalar is_equal ops whose accum_out gives counts for free).

  Per voxel chunk (4 chunks of 1024 voxel-cols x 128 partitions):
    e_c   = exp(x_c)                      ACT (single exp+ln table set, one
                                          manually placed ACT table load)
    s     = sum_c e_c                     PE identity-stationary PSUM matmuls
    lse   = ln(s)                         ACT
    m_c   = (lab == c)                    DVE tensor_scalar @4x, counts via
                                          accum_out
    gp_c  = m_c * x_c, lp_c = m_c * lse   DVE tensor_tensor @2x
    S_g[c], S_lse[c], sum(lse)            PE selector-stationary matmuls into
                                          one persistent [16,512] PSUM bank
  Final: ACT Copy+accum folds [16,512] -> [16,1]; host combines tiny
  per-core partials (class 7 lse-sum derived from the global lse sum).
"""

import numpy as np
import ml_dtypes

import concourse.bass as bass
import concourse.bacc as bacc
import concourse.mybir as mybir
from concourse import bass_utils, tile

BF16 = mybir.dt.bfloat16
F32 = mybir.dt.float32
NPBF16 = ml_dtypes.bfloat16

N_CORES = 8
B, C, D, H, W = 4, 8, 64, 128, 128
NCHUNK = 4          # voxel chunks per core
VCH = 1024          # voxel cols per chunk
XP_COLS = NCHUNK * C * VCH   # 32768
VOX_PER_CORE = 128 * NCHUNK * VCH  # 524288

_PROG_CACHE = {}

EXP = mybir.ActivationFunctionType.Exp
LN = mybir.ActivationFunctionType.Ln
COPY = mybir.ActivationFunctionType.Copy
EQ = mybir.AluOpType.is_equal
MUL = mybir.AluOpType.mult


def _build_program():
    nc = bacc.Bacc("TRN2", target_bir_lowering=False, debug=False)

    xp_in = nc.dram_tensor("xp", [128, XP_COLS], BF16, kind="ExternalInput")
    lab_in = nc.dram_tensor("lab", [128, NCHUNK * VCH], BF16, kind="ExternalInput")
    id_in = nc.dram_tensor("ident", [128, 128], BF16, kind="ExternalInput")
    es_in = nc.dram_tensor("esel", [128, 256], BF16, kind="ExternalInput")
    sums_out = nc.dram_tensor("sums", [16, 1], F32, kind="ExternalOutput")
    cnt_out = nc.dram_tensor("counts", [128, 32], F32, kind="ExternalOutput")

    with tile.TileContext(nc) as tc:
        with (
            tc.tile_pool(name="const", bufs=1) as cpool,
            tc.tile_pool(name="xp", bufs=1) as xpool,
            tc.tile_pool(name="e", bufs=2) as epool,
            tc.tile_pool(name="mask", bufs=3) as mpool,
            tc.tile_pool(name="gp", bufs=2) as gpool,
            tc.tile_pool(name="lp", bufs=3) as lpool,
            tc.tile_pool(name="psum", bufs=2, space="PSUM") as ppool,
            tc.tile_pool(name="psacc", bufs=1, space="PSUM") as papool,
        ):
            # One activation-table load serving both Exp and Ln
            # (set 6 = natural_log_exp_and_others); placed first on the
            # scalar engine so the compiler pass inserts no further loads.
            nc.scalar.add_instruction(
                mybir.InstLoadActFuncSet(
                    name="manual_actload", act_func_set_id=6, ins=[], outs=[]
                )
            )

            ident = cpool.tile([128, 128], BF16)
            esel = cpool.tile([128, 256], BF16)
            nc.sync.dma_start(ident[:], id_in[:])
            nc.sync.dma_start(esel[:], es_in[:])

            xp = xpool.tile([128, XP_COLS], BF16)
            lab = cpool.tile([128, NCHUNK * VCH], BF16)
            lse = cpool.tile([128, NCHUNK * VCH], BF16)
            cnts = cpool.tile([128, 32], F32)
            sums16 = cpool.tile([16, 1], F32)
            trash = cpool.tile([16, 512], BF16)

            # DMA order: lab chunk 0 first (unblocks DVE masks), then xp
            # chunk 0 halves (unblocks ACT exp), then the rest interleaved.
            ch = lambda k: slice(8192 * k, 8192 * (k + 1))
            lch = lambda k: slice(VCH * k, VCH * (k + 1))
            nc.sync.dma_start(lab[:, lch(0)], lab_in[:, lch(0)])
            nc.sync.dma_start(lab[:, lch(1)], lab_in[:, lch(1)])
            nc.sync.dma_start(xp[:, 8192 * 0 : 8192 * 0 + 4096], xp_in[:, 0:4096])
            nc.sync.dma_start(xp[:, 4096:8192], xp_in[:, 4096:8192])
            nc.sync.dma_start(lab[:, lch(2)], lab_in[:, lch(2)])
            nc.sync.dma_start(lab[:, lch(3)], lab_in[:, lch(3)])
            nc.sync.dma_start(xp[:, ch(1)], xp_in[:, ch(1)])
            nc.sync.dma_start(xp[:, ch(2)], xp_in[:, ch(2)])
            nc.sync.dma_start(xp[:, ch(3)], xp_in[:, ch(3)])

            # PE pre-warm: harmless matmuls on the identity to flip the HAM
            # clock gate to 2.4 GHz before the real dependency chain needs PE.
            ps_warm = ppool.tile([128, 512], F32, tag="ps")
            for _ in range(24):
                nc.tensor.matmul(
                    ps_warm[:, 0:128], ident[:], ident[:], start=True, stop=True
                )

            # 128 accumulating matmuls feed ps_sums over the whole kernel;
            # start on the first, stop on the last.
            N_ACC = NCHUNK * (16 + 2 + 14)
            acc_idx = [0]

            def acc_mm(sel_idx, rhs):
                # accumulate column-sums of rhs into ps_sums row sel_idx
                i = acc_idx[0]
                acc_idx[0] = i + 1
                nc.tensor.matmul(
                    ps_sums[:],
                    esel[:, 16 * sel_idx : 16 * sel_idx + 16],
                    rhs,
                    start=(i == 0),
                    stop=(i == N_ACC - 1),
                    skip_group_check=True,
                )

            ps_sums = papool.tile([16, 512], F32)

            masks = {}
            emit_masks_done = [0]

            def emit_masks(k):
                m = mpool.tile([128, C * VCH], BF16, tag="m")
                masks[k] = m
                for c in range(C):
                    nc.vector.tensor_scalar(
                        m[:, VCH * c : VCH * (c + 1)],
                        lab[:, lch(k)],
                        float(c),
                        None,
                        EQ,
                        op1=mybir.AluOpType.add,
                        accum_out=cnts[:, 8 * k + c : 8 * k + c + 1],
                    )

            def emit_gprod(k):
                # masked logits for all 8 classes of chunk k (two halves)
                for h in range(2):
                    gp = gpool.tile([128, 4096], BF16, tag="gp")
                    sl = slice(4096 * h, 4096 * (h + 1))
                    nc.vector.tensor_tensor(
                        gp[:], masks[k][:, sl], xp[:, 8192 * k + 4096 * h : 8192 * k + 4096 * (h + 1)], MUL
                    )
                    for c4 in range(4):
                        c = 4 * h + c4
                        for q in range(2):
                            acc_mm(c, gp[:, 1024 * c4 + 512 * q : 1024 * c4 + 512 * q + 512])

            def emit_exp(k, split=False):
                e = epool.tile([128, C * VCH], BF16, tag="e")
                if split:
                    base = 8192 * k
                    nc.scalar.activation(e[:, 0:4096], xp[:, base : base + 4096], EXP)
                    nc.scalar.activation(
                        e[:, 4096:8192], xp[:, base + 4096 : base + 8192], EXP
                    )
                else:
                    nc.scalar.activation(e[:], xp[:, ch(k)], EXP)
                return e

            def emit_smm_ln(k, e):
                # s = sum_c exp, in two 512-col PSUM banks, then lse = ln(s)
                for q in range(2):
                    ps = ppool.tile([128, 512], F32, tag="ps")
                    for c in range(C):
                        nc.tensor.matmul(
                            ps[:],
                            ident[:],
                            e[:, VCH * c + 512 * q : VCH * c + 512 * q + 512],
                            start=(c == 0),
                            stop=(c == C - 1),
                        )
                    nc.scalar.activation(
                        lse[:, VCH * k + 512 * q : VCH * k + 512 * q + 512], ps[:], LN
                    )
                # global lse sum -> row 15
                for q in range(2):
                    acc_mm(15, lse[:, VCH * k + 512 * q : VCH * k + 512 * q + 512])

            def emit_lprod(k):
                # masked lse for classes 0..6 of chunk k
                for c in range(7):
                    lp = lpool.tile([128, VCH], BF16, tag="lp")
                    nc.vector.tensor_tensor(
                        lp[:], masks[k][:, VCH * c : VCH * (c + 1)], lse[:, lch(k)], MUL
                    )
                    for q in range(2):
                        acc_mm(8 + c, lp[:, 512 * q : 512 * q + 512])

            # ---- pipelined emission ----
            e0 = emit_exp(0, split=True)
            emit_masks(0)
            emit_gprod(0)
            emit_smm_ln(0, e0)
            e1 = emit_exp(1)
            emit_masks(1)
            emit_gprod(1)
            emit_smm_ln(1, e1)
            emit_masks(2)
            emit_lprod(0)
            e2 = emit_exp(2)
            emit_gprod(2)
            emit_smm_ln(2, e2)
            emit_lprod(1)
            e3 = emit_exp(3)
            emit_masks(3)
            emit_gprod(3)
            emit_smm_ln(3, e3)
            emit_lprod(2)
            emit_lprod(3)
            assert acc_idx[0] == N_ACC, acc_idx[0]

            # fold [16,512] -> [16,1] on ACT (Copy is in the loaded set)
            nc.scalar.activation(trash[:], ps_sums[:], COPY, accum_out=sums16[:, 0:1])

            nc.sync.dma_start(sums_out[:], sums16[:])
            nc.sync.dma_start(cnt_out[:], cnts[:])

    nc.compile()
    return nc


def _host_prep(output, labels):
    """Shard + lay out inputs per core (dtype cast and reshapes only)."""
    x = np.asarray(output)
    lab = np.asarray(labels).astype(np.int32)

    ident = np.eye(128, dtype=NPBF16)
    esel = np.zeros((128, 256), dtype=NPBF16)
    for i in range(16):
        esel[:, 16 * i + i] = 1.0

    in_maps = []
    for k in range(N_CORES):
        b, d0 = k // 2, 32 * (k % 2)
        xc = x[b, :, d0 : d0 + 32]                      # [8, 32, 128, 128]
        xc = xc.reshape(C, 128, NCHUNK, VCH)            # [c, p, k, v]
        xp = np.ascontiguousarray(
            xc.transpose(1, 2, 0, 3), dtype=NPBF16
        ).reshape(128, XP_COLS)                         # [p, k*8192 + c*1024 + v]
        lc = lab[b, 0, d0 : d0 + 32].reshape(128, NCHUNK * VCH).astype(NPBF16)
        in_maps.append(
            {"xp": xp, "lab": lc, "ident": ident, "esel": esel}
        )
    return in_maps


def _combine(results):
    """Host gather: fold tiny per-core partials to the final scalar."""
    S_g = np.zeros(8, dtype=np.float64)
    S_lse = np.zeros(8, dtype=np.float64)
    cnt = np.zeros(8, dtype=np.float64)
    for r in results:
        s16 = np.asarray(r["sums"], dtype=np.float64)[:, 0]
        S_g += s16[0:8]
        lse7 = s16[8:15]
        S_lse[:7] += lse7
        S_lse[7] += s16[15] - lse7.sum()
        cnt += (
            np.asarray(r["counts"], dtype=np.float64)
            .reshape(128, NCHUNK, 8)
            .sum(axis=(0, 1))
        )
    sums = S_lse - S_g
    present = cnt > 0
    class_means = sums / np.maximum(cnt, 1.0)
    n_valid = present.sum()
    loss = np.where(present, class_means, 0.0).sum() / n_valid
    return np.float32(loss)


def run(inputs_maps=None, trace=False, **inputs):
    if "nc" not in _PROG_CACHE:
        _PROG_CACHE["nc"] = _build_program()
    nc = _PROG_CACHE["nc"]
    in_maps = inputs_maps if inputs_maps is not None else _host_prep(**inputs)
    res = bass_utils.run_bass_kernel_spmd(
        nc, in_maps, list(range(N_CORES)), trace=trace
    )
    return res


def kernel(output, labels):
    res = run(output=output, labels=labels)
    return _combine(res.results)


# revision 8
# speedup vs baseline: 1.2434x; 1.2434x over previous
"""Class-balanced cross-entropy loss kernel for Trainium2 (8 NeuronCores).

Problem: output [4,8,64,128,128] f32 logits, labels [4,1,64,128,128] int
(values 0..7).  loss = mean over present classes of (per-class mean CE).

Design ("plane layout", data-parallel over voxels, 524288 voxels/core):
  Host ships x as 8 class-planes in a voxel-major layout (bf16, 8.4MB/core)
  plus labels (bf16, 1MB/core).  No one-hot is shipped; per-class masks are
  generated on-device by 4x-rate tensor_scalar is_equal ops.  Per-class
  counts are a label-only quantity and are folded on the host (bincount).

  Layout: xp[p, k, c, v] — 4 voxel chunks (k) x 8 class planes (c) x 1024
  voxel cols (v) per 128 partitions.  lab[p, k, v] in the same voxel order.

  Per chunk k:
    e    = exp(xp[k])            ACT (one manually placed table load serves
                                 both Exp and Ln: set natural_log_exp)
    s    = sum_c e[c]            PE identity-stationary PSUM matmuls
    lse  = ln(s)                 ACT
    m    = (lab == c) per class  DVE tensor_scalar @4x (strided per-class out)
    gp   = m * xp[k]             DVE tensor_tensor @2x (all 8 classes at once)
    lp_c = m_c * lse             DVE tensor_tensor @2x (classes 0..6)
    S_g[c], S_lse[c], sum(lse)   PE selector-stationary matmuls accumulated
                                 into one persistent [16,512] PSUM bank
  Final: ACT Copy+accum folds [16,512] -> [16,1]; host combines the tiny
  per-core partials (class-7 lse sum derived from the global lse sum).
"""

import numpy as np
import ml_dtypes

import concourse.bass as bass
import concourse.bacc as bacc
import concourse.mybir as mybir
from concourse import bass_utils, tile

BF16 = mybir.dt.bfloat16
F32 = mybir.dt.float32
NPBF16 = ml_dtypes.bfloat16

N_CORES = 8
B, C, D, H, W = 4, 8, 64, 128, 128
NCHUNK = 4          # voxel chunks per core
VCH = 1024          # voxel cols per chunk
XP_COLS = NCHUNK * C * VCH   # 32768
VOX_PER_CORE = 128 * NCHUNK * VCH  # 524288

_PROG_CACHE = {}

EXP = mybir.ActivationFunctionType.Exp
LN = mybir.ActivationFunctionType.Ln
COPY = mybir.ActivationFunctionType.Copy
EQ = mybir.AluOpType.is_equal
MUL = mybir.AluOpType.mult


def _build_program():
    nc = bacc.Bacc("TRN2", target_bir_lowering=False, debug=False)

    xp_in = nc.dram_tensor("xp", [128, NCHUNK, C, VCH], BF16, kind="ExternalInput")
    lab_in = nc.dram_tensor("lab", [128, NCHUNK, VCH], BF16, kind="ExternalInput")
    id_in = nc.dram_tensor("ident", [128, 128], BF16, kind="ExternalInput")
    es_in = nc.dram_tensor("esel", [128, 256], BF16, kind="ExternalInput")
    sums_out = nc.dram_tensor("sums", [16, 1], F32, kind="ExternalOutput")

    with tile.TileContext(nc) as tc:
        with (
            tc.tile_pool(name="const", bufs=1) as cpool,
            tc.tile_pool(name="xp", bufs=1) as xpool,
            tc.tile_pool(name="e", bufs=2) as epool,
            tc.tile_pool(name="gp", bufs=1) as gpool,
            tc.tile_pool(name="lp", bufs=2) as lpool,
            tc.tile_pool(name="psum", bufs=2, space="PSUM") as ppool,
            tc.tile_pool(name="psacc", bufs=1, space="PSUM") as papool,
        ):
            # One activation-table load serving both Exp and Ln
            # (set 6 = natural_log_exp_and_others); placed first on the
            # scalar engine so the compiler pass inserts no further loads.
            nc.scalar.add_instruction(
                mybir.InstLoadActFuncSet(
                    name="manual_actload", act_func_set_id=6, ins=[], outs=[]
                )
            )

            ident = cpool.tile([128, 128], BF16)
            esel = cpool.tile([128, 256], BF16)
            nc.sync.dma_start(ident[:], id_in[:])
            nc.sync.dma_start(esel[:], es_in[:])

            xp = xpool.tile([128, NCHUNK, C, VCH], BF16)
            lab = cpool.tile([128, NCHUNK, VCH], BF16)
            masks = cpool.tile([128, NCHUNK, C, VCH], BF16)
            lse = cpool.tile([128, NCHUNK, VCH], BF16)
            sums16 = cpool.tile([16, 1], F32)
            trash = cpool.tile([16, 512], BF16)

            # DMA order: labels first (unblock mask gen), then xp chunks.
            nc.sync.dma_start(lab[:, 0:2], lab_in[:, 0:2])
            nc.sync.dma_start(lab[:, 2:4], lab_in[:, 2:4])
            nc.sync.dma_start(xp[:, 0, 0:4], xp_in[:, 0, 0:4])
            nc.sync.dma_start(xp[:, 0, 4:8], xp_in[:, 0, 4:8])
            for k in range(1, NCHUNK):
                nc.sync.dma_start(xp[:, k], xp_in[:, k])

            # PE pre-warm: harmless matmuls to flip the HAM clock gate to
            # 2.4 GHz before the real dependency chain reaches PE.
            ps_warm = ppool.tile([128, 512], F32, tag="ps")
            for _ in range(24):
                nc.tensor.matmul(
                    ps_warm[:, 0:128], ident[:], ident[:], start=True, stop=True
                )

            ps_sums = papool.tile([16, 512], F32)
            N_ACC = NCHUNK * (16 + 14 + 2)
            acc_idx = [0]

            def acc_mm(sel_idx, rhs):
                # accumulate column-sums of rhs into ps_sums row sel_idx
                i = acc_idx[0]
                acc_idx[0] = i + 1
                nc.tensor.matmul(
                    ps_sums[:],
                    esel[:, 16 * sel_idx : 16 * sel_idx + 16],
                    rhs,
                    start=(i == 0),
                    stop=(i == N_ACC - 1),
                    skip_group_check=True,
                )

            def emit_masks(half):
                # per-class masks for chunks [2*half, 2*half+2), all classes
                sl = slice(2 * half, 2 * half + 2)
                for c in range(C):
                    nc.vector.tensor_scalar(
                        masks[:, sl, c], lab[:, sl], float(c), None, EQ
                    )

            def emit_exp(k, split=False):
                e = epool.tile([128, C, VCH], BF16, tag="e")
                if split:
                    nc.scalar.activation(e[:, 0:4], xp[:, k, 0:4], EXP)
                    nc.scalar.activation(e[:, 4:8], xp[:, k, 4:8], EXP)
                else:
                    nc.scalar.activation(e[:], xp[:, k], EXP)
                return e

            def emit_gprod(k):
                # masked logits for all 8 classes of chunk k in one op
                gp = gpool.tile([128, C, VCH], BF16, tag="gp")
                nc.vector.tensor_tensor(gp[:], masks[:, k], xp[:, k], MUL)
                for c in range(C):
                    for q in range(2):
                        acc_mm(c, gp[:, c, 512 * q : 512 * q + 512])

            def emit_smm(k, e):
                # s = sum_c exp in two 512-col PSUM banks
                pss = []
                for q in range(2):
                    ps = ppool.tile([128, 512], F32, tag="ps")
                    for c in range(C):
                        nc.tensor.matmul(
                            ps[:],
                            ident[:],
                            e[:, c, 512 * q : 512 * q + 512],
                            start=(c == 0),
                            stop=(c == C - 1),
                        )
                    pss.append(ps)
                return pss

            def emit_ln(k, pss):
                for q in range(2):
                    nc.scalar.activation(
                        lse[:, k, 512 * q : 512 * q + 512], pss[q][:], LN
                    )

            def emit_lse_acc(k):
                for q in range(2):
                    acc_mm(15, lse[:, k, 512 * q : 512 * q + 512])

            def emit_lprod(k):
                # masked lse for classes 0..6 of chunk k
                for c in range(7):
                    lp = lpool.tile([128, VCH], BF16, tag="lp")
                    nc.vector.tensor_tensor(lp[:], masks[:, k, c], lse[:, k], MUL)
                    for q in range(2):
                        acc_mm(8 + c, lp[:, 512 * q : 512 * q + 512])

            # ---- pipelined emission ----
            emit_masks(0)
            e0 = emit_exp(0, split=True)
            emit_gprod(0)
            ps0 = emit_smm(0, e0)
            e1 = emit_exp(1)
            emit_masks(1)
            emit_gprod(1)
            emit_ln(0, ps0)
            emit_lse_acc(0)
            ps1 = emit_smm(1, e1)
            emit_lprod(0)
            e2 = emit_exp(2)
            emit_gprod(2)
            emit_ln(1, ps1)
            emit_lse_acc(1)
            ps2 = emit_smm(2, e2)
            emit_lprod(1)
            e3 = emit_exp(3)
            emit_gprod(3)
            emit_ln(2, ps2)
            emit_lse_acc(2)
            ps3 = emit_smm(3, e3)
            emit_lprod(2)
            emit_ln(3, ps3)
            emit_lse_acc(3)
            emit_lprod(3)
            assert acc_idx[0] == N_ACC, acc_idx[0]

            # fold [16,512] -> [16,1] on ACT (Copy is in the loaded set)
            nc.scalar.activation(trash[:], ps_sums[:], COPY, accum_out=sums16[:, 0:1])

            nc.sync.dma_start(sums_out[:], sums16[:])

    nc.compile()
    return nc


def _host_prep(output, labels):
    """Shard + lay out inputs per core (dtype cast and reshapes only)."""
    x = np.asarray(output)
    lab = np.asarray(labels).astype(np.int32)

    ident = np.eye(128, dtype=NPBF16)
    esel = np.zeros((128, 256), dtype=NPBF16)
    for i in range(16):
        esel[:, 16 * i + i] = 1.0

    in_maps = []
    for k in range(N_CORES):
        b, d0 = k // 2, 32 * (k % 2)
        xc = x[b, :, d0 : d0 + 32]                      # [8, 32, 128, 128]
        xc = xc.reshape(C, 128, NCHUNK, VCH)            # [c, p, k, v]
        xp = np.ascontiguousarray(
            xc.transpose(1, 2, 0, 3), dtype=NPBF16
        )                                               # [p, k, c, v]
        lc = lab[b, 0, d0 : d0 + 32].reshape(128, NCHUNK, VCH).astype(NPBF16)
        in_maps.append({"xp": xp, "lab": lc, "ident": ident, "esel": esel})
    return in_maps


def _combine(results, counts):
    """Host gather: fold tiny per-core partials to the final scalar."""
    S_g = np.zeros(8, dtype=np.float64)
    S_lse = np.zeros(8, dtype=np.float64)
    for r in results:
        s16 = np.asarray(r["sums"], dtype=np.float64)[:, 0]
        S_g += s16[0:8]
        lse7 = s16[8:15]
        S_lse[:7] += lse7
        S_lse[7] += s16[15] - lse7.sum()
    cnt = counts.astype(np.float64)
    sums = S_lse - S_g
    present = cnt > 0
    class_means = sums / np.maximum(cnt, 1.0)
    n_valid = present.sum()
    loss = np.where(present, class_means, 0.0).sum() / n_valid
    return np.float32(loss)


def run(inputs_maps=None, trace=False, **inputs):
    if "nc" not in _PROG_CACHE:
        _PROG_CACHE["nc"] = _build_program()
    nc = _PROG_CACHE["nc"]
    in_maps = inputs_maps if inputs_maps is not None else _host_prep(**inputs)
    res = bass_utils.run_bass_kernel_spmd(
        nc, in_maps, list(range(N_CORES)), trace=trace
    )
    return res


def kernel(output, labels):
    res = run(output=output, labels=labels)
    counts = np.bincount(np.asarray(labels).ravel().astype(np.int64), minlength=C)
    return _combine(res.results, counts)
